# revision 11
# baseline (speedup 1.0000x reference)
"""GroupedQueryAttention Bass kernel for 8 Trainium2 NeuronCores.

Sharding: 8 devices = 2 batches x 4 sequence-quarters.
Device d handles batch b=d//4, query rows [512*i, 512*(i+1)) with i=d%4.

v2: all matmul operands in BF16 (enables FastWeightLoad -> dense PE stream
that keeps the HAM clock gate at 2.4GHz; the fp32r baseline ran the PE cold
at 1.2GHz for 93% of the kernel).  Weights are pre-tiled on the host into
contiguous per-partition layouts so each weight is 1-4 large DMAs instead of
hundreds of strided 64KB ones.  Exp is batched 2 k-chunks per activation
instruction, and the attention inner loop is software-pipelined (scores for
group cg issue before AV/ones of group cg-1) so the PE FIFO never
head-of-line blocks on ScalarE's exp.

Per device:
  - K/V projection for the local 512-row slice (+RoPE on K, V transposed to
    s-major), AllGather over the 4 devices of the batch -> full-sequence K^T
    and V.  Q projection (16 heads) overlaps the collective.
  - Attention in transposed orientation: scores^T chunks from
    matmul(lhsT=k^T, rhs=q^T); exp on ScalarE (scale + sink bias fused);
    out^T accumulates matmul(lhsT=v, rhs=P^T); softmax denominators via
    matmul(lhsT=ones); normalization folded into the PSUM drain.
  - o_proj consumes out^T as lhsT with resident Wo; each device owns its
    [512, 2048] output rows -> host concatenates.
"""

from contextlib import ExitStack

import numpy as np
from ml_dtypes import bfloat16

import concourse.bass as bass
import concourse.tile as tile
from concourse import bacc, mybir
from concourse.bass_utils import run_bass_kernel_spmd
from concourse.masks import make_identity

F32 = mybir.dt.float32
BF16 = mybir.dt.bfloat16
AF = mybir.ActivationFunctionType
ALU = mybir.AluOpType

# Problem dims (hardcoded per contract)
B = 2
S = 2048
E = 2048
HQ = 16
HKV = 4
D = 128
REP = HQ // HKV          # 4 q-heads per kv head
NDEV = 8
DPB = 4                  # devices per batch
SQ = S // DPB            # 512 local query rows
EC = E // 128            # 16 contraction chunks
SKC = S // 128           # 16 key chunks
GC = 2                   # k-chunks per exp group
NG = SKC // GC           # 8 exp groups per head
SCALE = 1.0 / float(np.sqrt(D))

_CACHE = {}


def _build(sinks, with_bias_qkv, with_bias_o):
    nc = bacc.Bacc("TRN2", target_bir_lowering=False, debug=False, num_devices=NDEV)

    xT = nc.dram_tensor("xT", [128, EC * SQ], BF16, kind="ExternalInput").ap()
    wq = nc.dram_tensor("wq", [128, HQ * EC * 128], BF16, kind="ExternalInput").ap()
    wk = nc.dram_tensor("wk", [128, HKV * EC * 128], BF16, kind="ExternalInput").ap()
    wv = nc.dram_tensor("wv", [128, HKV * EC * 128], BF16, kind="ExternalInput").ap()
    wo = nc.dram_tensor("wo", [128, HQ * E], BF16, kind="ExternalInput").ap()
    cosT = nc.dram_tensor("cosT", [D // 2, SQ], F32, kind="ExternalInput").ap()
    sinT = nc.dram_tensor("sinT", [D // 2, SQ], F32, kind="ExternalInput").ap()
    if with_bias_qkv:
        # laid out [D, H] so a column is the per-partition bias of one head
        bqd = nc.dram_tensor("bqd", [D, HQ], F32, kind="ExternalInput").ap()
        bkd = nc.dram_tensor("bkd", [D, HKV], F32, kind="ExternalInput").ap()
        bvd = nc.dram_tensor("bvd", [D, HKV], F32, kind="ExternalInput").ap()
    if with_bias_o:
        bod = nc.dram_tensor("bod", [1, E], F32, kind="ExternalInput").ap()
    out = nc.dram_tensor("out", [SQ, E], F32, kind="ExternalOutput").ap()

    with tile.TileContext(nc) as tc, ExitStack() as es:
        _emit(tc, es, locals(), sinks, with_bias_qkv, with_bias_o)
    nc.compile()
    return nc


def _emit(tc, es, t, sinks, with_bias_qkv, with_bias_o):
    nc = tc.nc
    xT, wq, wk, wv, wo = t["xT"], t["wq"], t["wk"], t["wv"], t["wo"]
    cosT, sinT, out = t["cosT"], t["sinT"], t["out"]

    # ---------- persistent pools ----------
    const_pool = es.enter_context(tc.tile_pool(name="const", bufs=1))
    dram = es.enter_context(tc.tile_pool(name="dram", bufs=1, space="DRAM"))

    ident_f = const_pool.tile([128, 128], F32, tag="ident_f")
    make_identity(nc, ident_f[:])
    ident = const_pool.tile([128, 128], BF16, tag="ident")
    nc.vector.tensor_copy(ident[:], ident_f[:])
    ones_f = const_pool.tile([128, 1], F32, tag="ones_f")
    nc.vector.memset(ones_f[:], 1.0)
    ones = const_pool.tile([128, 1], BF16, tag="ones")
    nc.vector.tensor_copy(ones[:], ones_f[:])

    if with_bias_qkv:
        bq_sb = const_pool.tile([D, HQ], F32, tag="bq")
        nc.sync.dma_start(bq_sb[:], t["bqd"])
        bk_sb = const_pool.tile([D, HKV], F32, tag="bk")
        nc.sync.dma_start(bk_sb[:], t["bkd"])
        bv_sb = const_pool.tile([D, HKV], F32, tag="bv")
        nc.sync.dma_start(bv_sb[:], t["bvd"])

    sinks_sb = const_pool.tile([128, HQ], F32, tag="sinks")
    for _h in range(HQ):
        nc.vector.memset(sinks_sb[:, _h : _h + 1], float(sinks[_h]))

    q_sb = const_pool.tile([128, HQ * SQ], BF16, tag="q_sb")        # q^T, rope'd
    attn_sb = const_pool.tile([128, HQ * SQ], BF16, tag="attn_sb")  # out^T per head

    # Per-kv-head-pair collective buffers: pair p covers kv heads {2p, 2p+1}.
    # Splitting the AllGather in two lets attention on q-heads 0-7 (kv pair 0)
    # start as soon as the first collective lands, hiding the second behind it.
    # Flat pair buffer: [0 : VOFS) = k^T for heads {2p,2p+1} as (hh, d, s);
    # [VOFS : 2*VOFS) = v s-major as (s, hh, d).
    VOFS = 2 * D * SQ
    kv_sl = [dram.tile([2 * VOFS], BF16, tag=f"kvsl{p}", name=f"kvsl{p}") for p in range(2)]
    kv_g = [dram.tile([DPB, 2 * VOFS], BF16, tag=f"kvg{p}", name=f"kvg{p}") for p in range(2)]

    def rope(dst, src_ps, n_heads, cos_t, sin_t, tmp_pool, bias_sb=None, head0=0):
        """dst/src: [128, n_heads*SQ]; halves along partitions. bias optional."""
        w = n_heads * SQ
        src = src_ps[:].rearrange("p (h s) -> p h s", h=n_heads)
        if bias_sb is not None:
            for j in range(n_heads):
                nc.vector.tensor_scalar_add(
                    src_ps[:, j * SQ : (j + 1) * SQ],
                    src_ps[:, j * SQ : (j + 1) * SQ],
                    bias_sb[:, head0 + j : head0 + j + 1],
                )
        dstv = dst[:].rearrange("p (h s) -> p h s", h=n_heads)
        cosb = cos_t[:, None, :].to_broadcast((64, n_heads, SQ))
        sinb = sin_t[:, None, :].to_broadcast((64, n_heads, SQ))
        q1 = src[0:64]
        q2 = src[64:128]
        m1 = tmp_pool.tile([64, w], F32, tag="m", name="m1")[:].rearrange("p (h s) -> p h s", h=n_heads)
        m2 = tmp_pool.tile([64, w], F32, tag="m", name="m2")[:].rearrange("p (h s) -> p h s", h=n_heads)
        nc.vector.tensor_tensor(m1, q1, cosb, ALU.mult)
        nc.vector.tensor_tensor(m2, q2, sinb, ALU.mult)
        nc.vector.tensor_tensor(dstv[0:64], m1, m2, ALU.subtract)
        m3 = tmp_pool.tile([64, w], F32, tag="m", name="m3")[:].rearrange("p (h s) -> p h s", h=n_heads)
        m4 = tmp_pool.tile([64, w], F32, tag="m", name="m4")[:].rearrange("p (h s) -> p h s", h=n_heads)
        nc.vector.tensor_tensor(m3, q2, cosb, ALU.mult)
        nc.vector.tensor_tensor(m4, q1, sinb, ALU.mult)
        nc.vector.tensor_tensor(dstv[64:128], m3, m4, ALU.add)

    # ---------- phase 1: local KV projection + rope + transpose + gather ----
    with (
        tc.tile_pool(name="p12", bufs=1) as p12,
        tc.tile_pool(name="proj_ps", bufs=3, space="PSUM") as proj_ps,
        tc.tile_pool(name="tr_ps", bufs=2, space="PSUM") as tr_ps,
        tc.tile_pool(name="rope_tmp", bufs=4) as rope_tmp,
        tc.tile_pool(name="vtr", bufs=4) as vtr,
    ):
        # Startup DMAs, ordered so the first K-proj matmul can issue ASAP:
        # pair-0 K weights + first half of x, then the rest.
        xT_sb = p12.tile([128, EC * SQ], BF16, tag="xT")
        wk_sb = p12.tile([128, HKV * EC * 128], BF16, tag="wk")
        wv_sb = p12.tile([128, HKV * EC * 128], BF16, tag="wv")
        PW = 2 * EC * 128  # one kv-head pair of weight columns
        XH = EC * SQ // 2
        nc.sync.dma_start(wk_sb[:, 0:PW], wk[:, 0:PW])
        nc.sync.dma_start(xT_sb[:, 0:XH], xT[:, 0:XH])
        nc.sync.dma_start(xT_sb[:, XH:], xT[:, XH:])
        nc.sync.dma_start(wv_sb[:, 0:PW], wv[:, 0:PW])
        cos_sb = p12.tile([64, SQ], F32, tag="cos")
        nc.sync.dma_start(cos_sb[:], cosT)
        sin_sb = p12.tile([64, SQ], F32, tag="sin")
        nc.sync.dma_start(sin_sb[:], sinT)
        nc.sync.dma_start(wk_sb[:, PW:], wk[:, PW:])
        nc.sync.dma_start(wv_sb[:, PW:], wv[:, PW:])
        wkview = wk_sb[:].rearrange("p (h c n) -> p h c n", c=EC, n=128)
        wvview = wv_sb[:].rearrange("p (h c n) -> p h c n", c=EC, n=128)
        xview = xT_sb[:].rearrange("p (c s) -> p c s", s=SQ)

        wq_sb = p12.tile([128, HQ * EC * 128], BF16, tag="wq")
        QW = 4 * EC * 128  # 4 heads per DMA
        for g4 in range(HQ // 4):
            nc.sync.dma_start(
                wq_sb[:, g4 * QW : (g4 + 1) * QW], wq[:, g4 * QW : (g4 + 1) * QW]
            )
        wqview = wq_sb[:].rearrange("p (h c n) -> p h c n", c=EC, n=128)

        k_sb = p12.tile([128, HKV * SQ], BF16, tag="k_sb")
        v_sb = p12.tile([128, HKV * SQ], BF16, tag="v_sb")
        for pair in range(2):
            # K projection for kv heads {2p, 2p+1} + rope
            ps = proj_ps.tile([128, 2 * SQ], F32, tag="proj")
            for j in range(2):
                h = pair * 2 + j
                for c in range(EC):
                    nc.tensor.matmul(
                        ps[:, j * SQ : (j + 1) * SQ],
                        wkview[:, h, c, :],
                        xview[:, c, :],
                        start=(c == 0),
                        stop=(c == EC - 1),
                    )
            rope(
                k_sb[:, pair * 2 * SQ : (pair + 1) * 2 * SQ],
                ps, 2, cos_sb, sin_sb, rope_tmp,
                bias_sb=(bk_sb if with_bias_qkv else None), head0=pair * 2,
            )
            # V projection for the pair
            ps2 = proj_ps.tile([128, 2 * SQ], F32, tag="proj")
            for j in range(2):
                h = pair * 2 + j
                for c in range(EC):
                    nc.tensor.matmul(
                        ps2[:, j * SQ : (j + 1) * SQ],
                        wvview[:, h, c, :],
                        xview[:, c, :],
                        start=(c == 0),
                        stop=(c == EC - 1),
                    )
            if with_bias_qkv:
                for j in range(2):
                    nc.vector.tensor_scalar_add(
                        ps2[:, j * SQ : (j + 1) * SQ],
                        ps2[:, j * SQ : (j + 1) * SQ],
                        bv_sb[:, pair * 2 + j : pair * 2 + j + 1],
                    )
            nc.vector.tensor_copy(
                v_sb[:, pair * 2 * SQ : (pair + 1) * 2 * SQ], ps2[:]
            )
            # k^T slice out
            kreg = kv_sl[pair][0:VOFS].rearrange("(hd s) -> hd s", s=SQ)
            vreg = kv_sl[pair][VOFS : 2 * VOFS].rearrange("(s g d) -> s g d", g=2, d=128)
            for hh in range(2):
                h = pair * 2 + hh
                nc.sync.dma_start(
                    kreg[hh * 128 : (hh + 1) * 128, :],
                    k_sb[:, h * SQ : (h + 1) * SQ],
                )
            # v: transpose [d, s-block] -> [s-block, d], write s-major slice
            for hh in range(2):
                h = pair * 2 + hh
                for sc in range(SQ // 128):
                    tp = tr_ps.tile([128, 128], BF16, tag="trp")
                    nc.tensor.transpose(
                        tp[:], v_sb[:, h * SQ + sc * 128 : h * SQ + (sc + 1) * 128], ident[:]
                    )
                    ts_ = vtr.tile([128, 128], BF16, tag="vts")
                    nc.vector.tensor_copy(ts_[:], tp[:])
                    nc.sync.dma_start(
                        vreg[sc * 128 : (sc + 1) * 128, hh, :],
                        ts_[:],
                    )
            nc.gpsimd.collective_compute(
                "AllGather",
                ALU.bypass,
                ins=[kv_sl[pair][:].opt()],
                outs=[kv_g[pair][:].opt()],
                replica_groups=[[0, 1, 2, 3], [4, 5, 6, 7]],
            )

        # ---------- phase 2: Q projection + rope (overlaps collectives) -----
        for g in range(HQ // 2):
            ps = proj_ps.tile([128, 2 * SQ], F32, tag="proj")
            for j in range(2):
                h = g * 2 + j
                for c in range(EC):
                    nc.tensor.matmul(
                        ps[:, j * SQ : (j + 1) * SQ],
                        wqview[:, h, c, :],
                        xview[:, c, :],
                        start=(c == 0),
                        stop=(c == EC - 1),
                    )
            rope(
                q_sb[:, g * 2 * SQ : (g + 1) * 2 * SQ],
                ps, 2, cos_sb, sin_sb, rope_tmp,
                bias_sb=(bq_sb if with_bias_qkv else None), head0=g * 2,
            )

    # ---------- phase 3: attention ----------
    with (
        tc.tile_pool(name="kv_all", bufs=1) as kv_all,
        tc.tile_pool(name="wo_pool", bufs=1) as wo_pool,
        ExitStack() as attn_es,
    ):
        # Wo resident for phase 4; DMA streams during the collective wait.
        wo_sb = wo_pool.tile([128, HQ * E], BF16, tag="wo_sb")
        OW = 4 * E
        for g4 in range(HQ // 4):
            nc.sync.dma_start(
                wo_sb[:, g4 * OW : (g4 + 1) * OW], wo[:, g4 * OW : (g4 + 1) * OW]
            )
        woview = wo_sb[:].rearrange("p (h e) -> p h e", e=E)

        sc_ps = attn_es.enter_context(tc.tile_pool(name="sc_ps", bufs=2, space="PSUM"))
        out_ps = attn_es.enter_context(tc.tile_pool(name="out_ps", bufs=2, space="PSUM"))
        sum_ps = attn_es.enter_context(tc.tile_pool(name="sum_ps", bufs=2, space="PSUM"))
        p_pool = attn_es.enter_context(tc.tile_pool(name="p_pool", bufs=4))
        den_pool = attn_es.enter_context(tc.tile_pool(name="den_pool", bufs=3))

        # full-sequence K^T and V per kv head
        k_all = kv_all.tile([128, HKV * S], BF16, tag="k_all")   # [d, h*S + sk]
        v_all = kv_all.tile([128, HKV * S], BF16, tag="v_all")   # [s%128, h*S + c*128 + d]
        for pair in range(2):
            for hh in range(2):
                h = pair * 2 + hh
                for si in range(DPB):
                    nc.sync.dma_start(
                        k_all[:, h * S + si * SQ : h * S + (si + 1) * SQ],
                        kv_g[pair][si, hh * VOFS // 2 : (hh + 1) * VOFS // 2].rearrange(
                            "(d s) -> d s", s=SQ
                        ),
                    )
                    nc.sync.dma_start(
                        v_all[:, h * S + si * SQ : h * S + (si + 1) * SQ].rearrange(
                            "p (c d) -> p c d", d=128
                        ),
                        kv_g[pair][si, VOFS : 2 * VOFS].rearrange(
                            "(c q g d) -> q c g d", q=128, g=2, d=128
                        )[:, :, hh, :],
                    )

        for h in range(HQ):
            kh = h // REP
            op = out_ps.tile([128, SQ], F32, tag="outp")
            sp = sum_ps.tile([1, SQ], F32, tag="sump")
            pts = [None] * NG

            def emit_av(g):
                for j in range(GC):
                    c = g * GC + j
                    nc.tensor.matmul(
                        op[:],
                        v_all[:, kh * S + c * 128 : kh * S + (c + 1) * 128],
                        pts[g][:, j * 512 : (j + 1) * 512],
                        start=(c == 0),
                        stop=(c == SKC - 1),
                        skip_group_check=True,
                    )
                    nc.tensor.matmul(
                        sp[:],
                        ones[:],
                        pts[g][:, j * 512 : (j + 1) * 512],
                        start=(c == 0),
                        stop=(c == SKC - 1),
                        skip_group_check=True,
                    )

            for cg in range(NG):
                scp = sc_ps.tile([128, GC * 512], F32, tag="scp")
                for j in range(GC):
                    c = cg * GC + j
                    nc.tensor.matmul(
                        scp[:, j * 512 : (j + 1) * 512],
                        k_all[:, kh * S + c * 128 : kh * S + (c + 1) * 128],
                        q_sb[:, h * SQ : (h + 1) * SQ],
                        start=True,
                        stop=True,
                    )
                pt = p_pool.tile([128, GC * 512], BF16, tag="pt")
                nc.scalar.activation(pt[:], scp[:], AF.Exp, bias=sinks_sb[:, h : h + 1], scale=SCALE)
                pts[cg] = pt
                if cg >= 1:
                    emit_av(cg - 1)
            emit_av(NG - 1)

            rs = den_pool.tile([1, SQ], F32, tag="rs")
            nc.vector.reciprocal(rs[:], sp[:])
            den = den_pool.tile([128, SQ], F32, tag="den")
            nc.gpsimd.partition_broadcast(den[:], rs[:])
            nc.vector.tensor_tensor(
                attn_sb[:, h * SQ : (h + 1) * SQ], op[:], den[:], ALU.mult
            )

        # ---------- phase 4: o_proj ----------
        attn_es.close()
        with (
            tc.tile_pool(name="o_ps", bufs=2, space="PSUM") as o_ps,
            tc.tile_pool(name="o_sb", bufs=3) as o_sb_pool,
        ):
            if with_bias_o:
                bo_sb = const_pool.tile([1, E], F32, tag="bo")
                nc.sync.dma_start(bo_sb[:], t["bod"])
                bo_b = const_pool.tile([128, E], F32, tag="bo_b")
                nc.gpsimd.partition_broadcast(bo_b[:], bo_sb[:])
            for et in range(4):
                for sqc in range(SQ // 128):
                    ps = o_ps.tile([128, 512], F32, tag="ops")
                    for hd in range(HQ):
                        nc.tensor.matmul(
                            ps[:],
                            attn_sb[:, hd * SQ + sqc * 128 : hd * SQ + (sqc + 1) * 128],
                            woview[:, hd, et * 512 : (et + 1) * 512],
                            start=(hd == 0),
                            stop=(hd == HQ - 1),
                        )
                    ot = o_sb_pool.tile([128, 512], F32, tag="osb")
                    if with_bias_o:
                        nc.vector.tensor_tensor(
                            ot[:], ps[:], bo_b[:, et * 512 : (et + 1) * 512], ALU.add
                        )
                    else:
                        nc.scalar.copy(ot[:], ps[:])
                    nc.sync.dma_start(
                        out[sqc * 128 : (sqc + 1) * 128, et * 512 : (et + 1) * 512],
                        ot[:],
                    )


RUN_KWARGS = {}


def kernel(x, sin, cos, Wq, bq, Wk, bk, Wv, bv, Wo, bo, sinks):
    x = np.asarray(x, dtype=np.float32)
    sin = np.asarray(sin, dtype=np.float32)
    cos = np.asarray(cos, dtype=np.float32)
    sinks = np.asarray(sinks, dtype=np.float32)
    with_bias_qkv = bool(np.any(bq) or np.any(bk) or np.any(bv))
    with_bias_o = bool(np.any(bo))

    key = (sinks.tobytes(), with_bias_qkv, with_bias_o)
    if key not in _CACHE:
        _CACHE[key] = _build(sinks, with_bias_qkv, with_bias_o)
    nc = _CACHE[key]

    def tile_w(W, H):
        # [E, H*128] -> [128, H*EC*128] with free index (h, c, n)
        W = np.asarray(W, dtype=np.float32)
        return np.ascontiguousarray(
            W.reshape(EC, 128, H, 128).transpose(1, 2, 0, 3).reshape(128, H * EC * 128)
        ).astype(bfloat16)

    wq_t = tile_w(Wq, HQ)
    wk_t = tile_w(Wk, HKV)
    wv_t = tile_w(Wv, HKV)
    # Wo [HQ*D, E] -> [128, HQ*E] with free index (hd, e)
    wo_t = np.ascontiguousarray(
        np.asarray(Wo, np.float32).reshape(HQ, 128, E).transpose(1, 0, 2).reshape(128, HQ * E)
    ).astype(bfloat16)

    in_maps = []
    for dev in range(NDEV):
        b, i = divmod(dev, DPB)
        sl = slice(SQ * i, SQ * (i + 1))
        xs = x[b, sl, :]  # [SQ, E]
        xT_t = np.ascontiguousarray(
            xs.T.reshape(EC, 128, SQ).transpose(1, 0, 2).reshape(128, EC * SQ)
        ).astype(bfloat16)
        m = {
            "xT": xT_t,
            "wq": wq_t,
            "wk": wk_t,
            "wv": wv_t,
            "wo": wo_t,
            "cosT": np.ascontiguousarray(cos[b, sl, :].T),
            "sinT": np.ascontiguousarray(sin[b, sl, :].T),
        }
        if with_bias_qkv:
            m["bqd"] = np.ascontiguousarray(np.asarray(bq, np.float32).reshape(HQ, D).T)
            m["bkd"] = np.ascontiguousarray(np.asarray(bk, np.float32).reshape(HKV, D).T)
            m["bvd"] = np.ascontiguousarray(np.asarray(bv, np.float32).reshape(HKV, D).T)
        if with_bias_o:
            m["bod"] = np.asarray(bo, np.float32).reshape(1, E)
        in_maps.append(m)

    res = run_bass_kernel_spmd(nc, in_maps, list(range(NDEV)), **RUN_KWARGS)
    kernel.last_result = res

    out = np.empty((B, S, E), dtype=np.float32)
    for dev in range(NDEV):
        b, i = divmod(dev, DPB)
        out[b, SQ * i : SQ * (i + 1), :] = res.results[dev]["out"]
    return out


# revision 16
# speedup vs baseline: 1.0576x; 1.0576x over previous
"""GroupedQueryAttention Bass kernel for 8 Trainium2 NeuronCores.

Sharding: 8 devices = 2 batches x 4 sequence-quarters.
Device d handles batch b=d//4, query rows [512*i, 512*(i+1)) with i=d%4.

v4: all matmul operands BF16 (FastWeightLoad + warm HAM clock; the fp32r
baseline ran the PE at 1.2GHz).  Weights pre-tiled host-side into contiguous
layouts (few large DMAs).  V projection runs x-stationary so V comes out
s-major directly -- no PE transposes, and each kv-head pair's K/V slice
ships to the collective buffer as 2 large DMAs.  The KV AllGather is split
into two per-pair collectives so attention on q-heads 0-7 only waits for the
first one.  Attention processes q-heads in pairs sharing one kv head: the
k/v chunk is the stationary operand for both heads' matmuls, exp is batched
[128,1024] per chunk with no bias (the additive 'sinks' term is constant
per head across q AND k, so softmax cancels it exactly), and both heads'
softmax denominators accumulate in one PSUM bank (partitions 0/32) so one
reciprocal serves the pair.  Normalization happens in SBUF after an
unnormalized bf16 drain, keeping PSUM pressure at 8 banks exactly.
"""

from contextlib import ExitStack

import numpy as np
from ml_dtypes import bfloat16

import concourse.bass as bass
import concourse.tile as tile
from concourse import bacc, mybir
from concourse.bass_utils import run_bass_kernel_spmd

F32 = mybir.dt.float32
BF16 = mybir.dt.bfloat16
AF = mybir.ActivationFunctionType
ALU = mybir.AluOpType

# Problem dims (hardcoded per contract)
B = 2
S = 2048
E = 2048
HQ = 16
HKV = 4
D = 128
REP = HQ // HKV          # 4 q-heads per kv head
NDEV = 8
DPB = 4                  # devices per batch
SQ = S // DPB            # 512 local query rows
EC = E // 128            # 16 contraction chunks
SKC = S // 128           # 16 key chunks
SCALE = 1.0 / float(np.sqrt(D))
VOFS = 2 * D * SQ        # bytes.. elems per region in a pair's collective buffer

_CACHE = {}


def _build(with_bias_qkv, with_bias_o):
    nc = bacc.Bacc("TRN2", target_bir_lowering=False, debug=False, num_devices=NDEV)

    xT = nc.dram_tensor("xT", [128, EC * SQ], BF16, kind="ExternalInput").ap()
    wq = nc.dram_tensor("wq", [128, HQ * EC * 128], BF16, kind="ExternalInput").ap()
    wk = nc.dram_tensor("wk", [128, HKV * EC * 128], BF16, kind="ExternalInput").ap()
    wv = nc.dram_tensor("wv", [128, HKV * EC * 128], BF16, kind="ExternalInput").ap()
    wo = nc.dram_tensor("wo", [128, HQ * E], BF16, kind="ExternalInput").ap()
    cosT = nc.dram_tensor("cosT", [D // 2, SQ], F32, kind="ExternalInput").ap()
    sinT = nc.dram_tensor("sinT", [D // 2, SQ], F32, kind="ExternalInput").ap()
    if with_bias_qkv:
        bqd = nc.dram_tensor("bqd", [D, HQ], F32, kind="ExternalInput").ap()
        bkd = nc.dram_tensor("bkd", [D, HKV], F32, kind="ExternalInput").ap()
        bvr = nc.dram_tensor("bvr", [1, HKV * D], F32, kind="ExternalInput").ap()
    if with_bias_o:
        bod = nc.dram_tensor("bod", [1, E], F32, kind="ExternalInput").ap()
    out = nc.dram_tensor("out", [SQ, E], F32, kind="ExternalOutput").ap()

    with tile.TileContext(nc) as tc, ExitStack() as es:
        _emit(tc, es, locals(), with_bias_qkv, with_bias_o)
    nc.compile()
    return nc


def _emit(tc, es, t, with_bias_qkv, with_bias_o):
    nc = tc.nc
    xT, wq, wk, wv, wo = t["xT"], t["wq"], t["wk"], t["wv"], t["wo"]
    cosT, sinT, out = t["cosT"], t["sinT"], t["out"]

    # ---------- persistent pools ----------
    const_pool = es.enter_context(tc.tile_pool(name="const", bufs=1))
    dram = es.enter_context(tc.tile_pool(name="dram", bufs=1, space="DRAM"))

    ones_f = const_pool.tile([128, 1], F32, tag="ones_f")
    nc.vector.memset(ones_f[:], 1.0)
    ones = const_pool.tile([128, 1], BF16, tag="ones")
    nc.vector.tensor_copy(ones[:], ones_f[:])

    if with_bias_qkv:
        bq_sb = const_pool.tile([D, HQ], F32, tag="bq")
        nc.sync.dma_start(bq_sb[:], t["bqd"])
        bk_sb = const_pool.tile([D, HKV], F32, tag="bk")
        nc.sync.dma_start(bk_sb[:], t["bkd"])
        bvr_sb = const_pool.tile([1, HKV * D], F32, tag="bvr")
        nc.sync.dma_start(bvr_sb[:], t["bvr"])
        bvb = const_pool.tile([128, HKV * D], F32, tag="bvb")
        nc.gpsimd.partition_broadcast(bvb[:], bvr_sb[:])
        bvbview = bvb[:].rearrange("p (h d) -> p h d", d=128)

    q_sb = const_pool.tile([128, HQ * SQ], BF16, tag="q_sb")        # q^T, rope'd
    attn_sb = const_pool.tile([128, HQ * SQ], BF16, tag="attn_sb")  # out^T per head

    # Flat per-pair collective buffer: [0:VOFS) = k^T as (hh, d, s);
    # [VOFS:2*VOFS) = v s-major as (s, hh, d).
    kv_sl = [dram.tile([2 * VOFS], BF16, tag=f"kvsl{p}", name=f"kvsl{p}") for p in range(2)]
    kv_g = [dram.tile([DPB, 2 * VOFS], BF16, tag=f"kvg{p}", name=f"kvg{p}") for p in range(2)]

    def rope(dst, src_ps, n_heads, cos_t, sin_t, tmp_pool, bias_sb=None, head0=0):
        """dst/src: [128, n_heads*SQ]; halves along partitions. bias optional."""
        w = n_heads * SQ
        src = src_ps[:].rearrange("p (h s) -> p h s", h=n_heads)
        if bias_sb is not None:
            for j in range(n_heads):
                nc.vector.tensor_scalar_add(
                    src_ps[:, j * SQ : (j + 1) * SQ],
                    src_ps[:, j * SQ : (j + 1) * SQ],
                    bias_sb[:, head0 + j : head0 + j + 1],
                )
        dstv = dst[:].rearrange("p (h s) -> p h s", h=n_heads)
        cosb = cos_t[:, None, :].to_broadcast((64, n_heads, SQ))
        sinb = sin_t[:, None, :].to_broadcast((64, n_heads, SQ))
        q1 = src[0:64]
        q2 = src[64:128]
        m1 = tmp_pool.tile([64, w], F32, tag="m", name="m1")[:].rearrange("p (h s) -> p h s", h=n_heads)
        m2 = tmp_pool.tile([64, w], F32, tag="m", name="m2")[:].rearrange("p (h s) -> p h s", h=n_heads)
        nc.vector.tensor_tensor(m1, q1, cosb, ALU.mult)
        nc.vector.tensor_tensor(m2, q2, sinb, ALU.mult)
        nc.vector.tensor_tensor(dstv[0:64], m1, m2, ALU.subtract)
        m3 = tmp_pool.tile([64, w], F32, tag="m", name="m3")[:].rearrange("p (h s) -> p h s", h=n_heads)
        m4 = tmp_pool.tile([64, w], F32, tag="m", name="m4")[:].rearrange("p (h s) -> p h s", h=n_heads)
        nc.vector.tensor_tensor(m3, q2, cosb, ALU.mult)
        nc.vector.tensor_tensor(m4, q1, sinb, ALU.mult)
        nc.vector.tensor_tensor(dstv[64:128], m3, m4, ALU.add)

    # ---------- phase 1: per-pair KV projection + gather ----
    with (
        tc.tile_pool(name="p12", bufs=1) as p12,
        tc.tile_pool(name="proj_ps", bufs=3, space="PSUM") as proj_ps,
        tc.tile_pool(name="v_ps", bufs=2, space="PSUM") as v_ps,
        tc.tile_pool(name="rope_tmp", bufs=4) as rope_tmp,
    ):
        # Startup DMAs, ordered so the first K-proj matmul can issue ASAP.
        xT_sb = p12.tile([128, EC * SQ], BF16, tag="xT")
        wk_sb = p12.tile([128, HKV * EC * 128], BF16, tag="wk")
        wv_sb = p12.tile([128, HKV * EC * 128], BF16, tag="wv")
        PW = 2 * EC * 128  # one kv-head pair of weight columns
        XH = EC * SQ // 2
        nc.sync.dma_start(wk_sb[:, 0:PW], wk[:, 0:PW])
        nc.sync.dma_start(xT_sb[:, 0:XH], xT[:, 0:XH])
        nc.sync.dma_start(xT_sb[:, XH:], xT[:, XH:])
        nc.sync.dma_start(wv_sb[:, 0:PW], wv[:, 0:PW])
        cos_sb = p12.tile([64, SQ], F32, tag="cos")
        nc.sync.dma_start(cos_sb[:], cosT)
        sin_sb = p12.tile([64, SQ], F32, tag="sin")
        nc.sync.dma_start(sin_sb[:], sinT)
        nc.sync.dma_start(wk_sb[:, PW:], wk[:, PW:])
        nc.sync.dma_start(wv_sb[:, PW:], wv[:, PW:])
        wkview = wk_sb[:].rearrange("p (h c n) -> p h c n", c=EC, n=128)
        wvview = wv_sb[:].rearrange("p (h c n) -> p h c n", c=EC, n=128)
        xview = xT_sb[:].rearrange("p (c s) -> p c s", s=SQ)

        wq_sb = p12.tile([128, HQ * EC * 128], BF16, tag="wq")
        QW = 4 * EC * 128  # 4 heads per DMA
        for g4 in range(HQ // 4):
            nc.sync.dma_start(
                wq_sb[:, g4 * QW : (g4 + 1) * QW], wq[:, g4 * QW : (g4 + 1) * QW]
            )
        wqview = wq_sb[:].rearrange("p (h c n) -> p h c n", c=EC, n=128)

        k_sb = p12.tile([128, HKV * SQ], BF16, tag="k_sb")
        v_loc = p12.tile([128, HKV * 4 * 128], BF16, tag="v_loc")  # (h, sc, d)
        vlview = v_loc[:].rearrange("p (h c d) -> p h c d", c=4, d=128)
        for pair in range(2):
            # K projection (weight-stationary) + rope for kv heads {2p, 2p+1}
            ps = proj_ps.tile([128, 2 * SQ], F32, tag="proj")
            for j in range(2):
                h = pair * 2 + j
                for c in range(EC):
                    nc.tensor.matmul(
                        ps[:, j * SQ : (j + 1) * SQ],
                        wkview[:, h, c, :],
                        xview[:, c, :],
                        start=(c == 0),
                        stop=(c == EC - 1),
                    )
            rope(
                k_sb[:, pair * 2 * SQ : (pair + 1) * 2 * SQ],
                ps, 2, cos_sb, sin_sb, rope_tmp,
                bias_sb=(bk_sb if with_bias_qkv else None), head0=pair * 2,
            )
            # V projection x-stationary: out [s-block, (hh, d)] is s-major
            for sc in range(4):
                vps = v_ps.tile([128, 2 * 128], F32, tag="vps")
                vpsv = vps[:].rearrange("p (h d) -> p h d", d=128)
                for c in range(EC):
                    nc.tensor.matmul(
                        vpsv,
                        xview[:, c, sc * 128 : (sc + 1) * 128],
                        wvview[:, pair * 2 : pair * 2 + 2, c, :],
                        start=(c == 0),
                        stop=(c == EC - 1),
                    )
                if with_bias_qkv:
                    nc.vector.tensor_tensor(
                        vpsv, vpsv,
                        bvbview[:, pair * 2 : pair * 2 + 2, :], ALU.add,
                    )
                nc.vector.tensor_copy(
                    vlview[:, pair * 2 : pair * 2 + 2, sc, :], vpsv
                )
            # ship the pair's K and V to the collective buffer: 2 big DMAs
            kreg = kv_sl[pair][0:VOFS].rearrange("(h d s) -> d h s", h=2, s=SQ)
            nc.sync.dma_start(
                kreg,
                k_sb[:, pair * 2 * SQ : (pair + 1) * 2 * SQ].rearrange(
                    "p (h s) -> p h s", h=2
                ),
            )
            vreg = kv_sl[pair][VOFS : 2 * VOFS].rearrange(
                "(c p h d) -> p c h d", p=128, h=2, d=128
            )
            for hh in range(2):
                nc.sync.dma_start(
                    vreg[:, :, hh, :],
                    vlview[:, pair * 2 + hh, :, :],
                )
            nc.gpsimd.collective_compute(
                "AllGather",
                ALU.bypass,
                ins=[kv_sl[pair][:].opt()],
                outs=[kv_g[pair][:].opt()],
                replica_groups=[[0, 1, 2, 3], [4, 5, 6, 7]],
            )

        # ---------- phase 2: Q projection + rope (overlaps collectives) -----
        for g in range(HQ // 2):
            ps = proj_ps.tile([128, 2 * SQ], F32, tag="proj")
            for j in range(2):
                h = g * 2 + j
                for c in range(EC):
                    nc.tensor.matmul(
                        ps[:, j * SQ : (j + 1) * SQ],
                        wqview[:, h, c, :],
                        xview[:, c, :],
                        start=(c == 0),
                        stop=(c == EC - 1),
                    )
            rope(
                q_sb[:, g * 2 * SQ : (g + 1) * 2 * SQ],
                ps, 2, cos_sb, sin_sb, rope_tmp,
                bias_sb=(bq_sb if with_bias_qkv else None), head0=g * 2,
            )

    # ---------- phase 3: attention ----------
    with (
        tc.tile_pool(name="kv_all", bufs=1) as kv_all,
        tc.tile_pool(name="wo_pool", bufs=1) as wo_pool,
        ExitStack() as attn_es,
    ):
        # Wo resident for phase 4; streams during the collective wait.
        wo_sb = wo_pool.tile([128, HQ * E], BF16, tag="wo_sb")
        OW = 4 * E
        for g4 in range(HQ // 4):
            nc.sync.dma_start(
                wo_sb[:, g4 * OW : (g4 + 1) * OW], wo[:, g4 * OW : (g4 + 1) * OW]
            )
        woview = wo_sb[:].rearrange("p (h e) -> p h e", e=E)

        sc_ps = attn_es.enter_context(tc.tile_pool(name="sc_ps", bufs=2, space="PSUM"))
        out_ps = attn_es.enter_context(tc.tile_pool(name="out_ps", bufs=1, space="PSUM"))
        sum_ps = attn_es.enter_context(tc.tile_pool(name="sum_ps", bufs=2, space="PSUM"))
        p_pool = attn_es.enter_context(tc.tile_pool(name="p_pool", bufs=4))
        den_pool = attn_es.enter_context(tc.tile_pool(name="den_pool", bufs=3))

        # full-sequence K^T and V per kv head
        k_all = kv_all.tile([128, HKV * S], BF16, tag="k_all")   # [d, h*S + sk]
        v_all = kv_all.tile([128, HKV * S], BF16, tag="v_all")   # [s%128, h*S + c*128 + d]
        for pair in range(2):
            for si in range(DPB):
                nc.sync.dma_start(
                    k_all[:, :].rearrange("p (h s) -> p h s", h=HKV)[
                        :, pair * 2 : pair * 2 + 2, si * SQ : (si + 1) * SQ
                    ],
                    kv_g[pair][si, 0:VOFS].rearrange("(h d s) -> d h s", h=2, s=SQ),
                )
                for hh in range(2):
                    nc.sync.dma_start(
                        v_all[:, :].rearrange("p (h c d) -> p h c d", h=HKV, d=128)[
                            :, pair * 2 + hh, si * 4 : (si + 1) * 4, :
                        ],
                        kv_g[pair][si, VOFS : 2 * VOFS].rearrange(
                            "(c p h d) -> p c h d", p=128, h=2, d=128
                        )[:, :, hh, :],
                    )

        for hp in range(HQ // 2):      # head pair: q-heads {2hp, 2hp+1}
            kh = hp // 2               # shared kv head
            h0 = 2 * hp
            op = out_ps.tile([128, 2 * SQ], F32, tag="outp")
            sps = [
                sum_ps.tile([1, SQ], F32, tag="sump", name=f"sp{j}") for j in range(2)
            ]
            pts = [None] * SKC

            def emit_av(c):
                vchunk = v_all[:, kh * S + c * 128 : kh * S + (c + 1) * 128]
                for j in range(2):
                    nc.tensor.matmul(
                        op[:, j * SQ : (j + 1) * SQ],
                        vchunk,
                        pts[c][:, j * SQ : (j + 1) * SQ],
                        start=(c == 0),
                        stop=(c == SKC - 1),
                        skip_group_check=True,
                    )
                for j in range(2):
                    nc.tensor.matmul(
                        sps[j][:],
                        ones[:],
                        pts[c][:, j * SQ : (j + 1) * SQ],
                        start=(c == 0),
                        stop=(c == SKC - 1),
                        skip_group_check=True,
                    )

            for c in range(SKC):
                kchunk = k_all[:, kh * S + c * 128 : kh * S + (c + 1) * 128]
                scp = sc_ps.tile([128, 2 * SQ], F32, tag="scp")
                for j in range(2):
                    nc.tensor.matmul(
                        scp[:, j * SQ : (j + 1) * SQ],
                        kchunk,
                        q_sb[:, (h0 + j) * SQ : (h0 + j + 1) * SQ],
                        start=True,
                        stop=True,
                    )
                pt = p_pool.tile([128, 2 * SQ], BF16, tag="pt")
                nc.scalar.activation(pt[:], scp[:], AF.Exp, scale=SCALE)
                pts[c] = pt
                if c >= 1:
                    emit_av(c - 1)
            emit_av(SKC - 1)

            # drain unnormalized out^T (frees op quickly), copy den sums out of
            # PSUM (frees sp banks), then normalize in SBUF
            un = attn_sb[:, h0 * SQ : (h0 + 2) * SQ]
            nc.vector.tensor_copy(un, op[:])
            dcs = []
            for j in range(2):
                dc = den_pool.tile([1, SQ], F32, tag="dc", name=f"dc{j}")
                nc.vector.tensor_copy(dc[:], sps[j][:])
                dcs.append(dc)
            for j in range(2):
                h = h0 + j
                rs = den_pool.tile([1, SQ], F32, tag="rs", name=f"rs{j}")
                nc.vector.reciprocal(rs[:], dcs[j][:])
                den = den_pool.tile([128, SQ], F32, tag="den", name=f"den{j}")
                nc.gpsimd.partition_broadcast(den[:], rs[:])
                nc.vector.tensor_tensor(
                    attn_sb[:, h * SQ : (h + 1) * SQ],
                    attn_sb[:, h * SQ : (h + 1) * SQ],
                    den[:], ALU.mult,
                )

        # ---------- phase 4: o_proj ----------
        attn_es.close()
        with (
            tc.tile_pool(name="o_ps", bufs=2, space="PSUM") as o_ps,
            tc.tile_pool(name="o_sb", bufs=3) as o_sb_pool,
        ):
            if with_bias_o:
                bo_sb = const_pool.tile([1, E], F32, tag="bo")
                nc.sync.dma_start(bo_sb[:], t["bod"])
                bo_b = const_pool.tile([128, E], F32, tag="bo_b")
                nc.gpsimd.partition_broadcast(bo_b[:], bo_sb[:])
            for et in range(4):
                for sqc in range(SQ // 128):
                    ps = o_ps.tile([128, 512], F32, tag="ops")
                    for hd in range(HQ):
                        nc.tensor.matmul(
                            ps[:],
                            attn_sb[:, hd * SQ + sqc * 128 : hd * SQ + (sqc + 1) * 128],
                            woview[:, hd, et * 512 : (et + 1) * 512],
                            start=(hd == 0),
                            stop=(hd == HQ - 1),
                        )
                    ot = o_sb_pool.tile([128, 512], F32, tag="osb")
                    if with_bias_o:
                        nc.vector.tensor_tensor(
                            ot[:], ps[:], bo_b[:, et * 512 : (et + 1) * 512], ALU.add
                        )
                    else:
                        nc.scalar.copy(ot[:], ps[:])
                    nc.sync.dma_start(
                        out[sqc * 128 : (sqc + 1) * 128, et * 512 : (et + 1) * 512],
                        ot[:],
                    )


RUN_KWARGS = {}


def kernel(x, sin, cos, Wq, bq, Wk, bk, Wv, bv, Wo, bo, sinks):
    x = np.asarray(x, dtype=np.float32)
    sin = np.asarray(sin, dtype=np.float32)
    cos = np.asarray(cos, dtype=np.float32)
    with_bias_qkv = bool(np.any(bq) or np.any(bk) or np.any(bv))
    with_bias_o = bool(np.any(bo))

    key = (with_bias_qkv, with_bias_o)
    if key not in _CACHE:
        _CACHE[key] = _build(with_bias_qkv, with_bias_o)
    nc = _CACHE[key]

    def tile_w(W, H):
        # [E, H*128] -> [128, H*EC*128] with free index (h, c, n)
        W = np.asarray(W, dtype=np.float32)
        return np.ascontiguousarray(
            W.reshape(EC, 128, H, 128).transpose(1, 2, 0, 3).reshape(128, H * EC * 128)
        ).astype(bfloat16)

    wq_t = tile_w(Wq, HQ)
    wk_t = tile_w(Wk, HKV)
    wv_t = tile_w(Wv, HKV)
    # Wo [HQ*D, E] -> [128, HQ*E] with free index (hd, e)
    wo_t = np.ascontiguousarray(
        np.asarray(Wo, np.float32).reshape(HQ, 128, E).transpose(1, 0, 2).reshape(128, HQ * E)
    ).astype(bfloat16)

    in_maps = []
    for dev in range(NDEV):
        b, i = divmod(dev, DPB)
        sl = slice(SQ * i, SQ * (i + 1))
        xs = x[b, sl, :]  # [SQ, E]
        xT_t = np.ascontiguousarray(
            xs.T.reshape(EC, 128, SQ).transpose(1, 0, 2).reshape(128, EC * SQ)
        ).astype(bfloat16)
        m = {
            "xT": xT_t,
            "wq": wq_t,
            "wk": wk_t,
            "wv": wv_t,
            "wo": wo_t,
            "cosT": np.ascontiguousarray(cos[b, sl, :].T),
            "sinT": np.ascontiguousarray(sin[b, sl, :].T),
        }
        if with_bias_qkv:
            m["bqd"] = np.ascontiguousarray(np.asarray(bq, np.float32).reshape(HQ, D).T)
            m["bkd"] = np.ascontiguousarray(np.asarray(bk, np.float32).reshape(HKV, D).T)
            m["bvr"] = np.asarray(bv, np.float32).reshape(1, HKV * D)
        if with_bias_o:
            m["bod"] = np.asarray(bo, np.float32).reshape(1, E)
        in_maps.append(m)

    res = run_bass_kernel_spmd(nc, in_maps, list(range(NDEV)), **RUN_KWARGS)
    kernel.last_result = res

    out = np.empty((B, S, E), dtype=np.float32)
    for dev in range(NDEV):
        b, i = divmod(dev, DPB)
        out[b, SQ * i : SQ * (i + 1), :] = res.results[dev]["out"]
    return out


# revision 21
# speedup vs baseline: 1.0659x; 1.0079x over previous
"""GroupedQueryAttention Bass kernel for 8 Trainium2 NeuronCores.

Sharding: 8 devices = 2 batches x 4 sequence-quarters.
Device d handles batch b=d//4, query rows [512*i, 512*(i+1)) with i=d%4.

v4: all matmul operands BF16 (FastWeightLoad + warm HAM clock; the fp32r
baseline ran the PE at 1.2GHz).  Weights pre-tiled host-side into contiguous
layouts (few large DMAs).  V projection runs x-stationary so V comes out
s-major directly -- no PE transposes, and each kv-head pair's K/V slice
ships to the collective buffer as 2 large DMAs.  The KV AllGather is split
into two per-pair collectives so attention on q-heads 0-7 only waits for the
first one.  Attention processes q-heads in pairs sharing one kv head: the
k/v chunk is the stationary operand for both heads' matmuls, exp is batched
[128,1024] per chunk with no bias (the additive 'sinks' term is constant
per head across q AND k, so softmax cancels it exactly), and both heads'
softmax denominators accumulate in one PSUM bank (partitions 0/32) so one
reciprocal serves the pair.  Normalization happens in SBUF after an
unnormalized bf16 drain, keeping PSUM pressure at 8 banks exactly.
"""

from contextlib import ExitStack

import numpy as np
from ml_dtypes import bfloat16

import concourse.bass as bass
import concourse.tile as tile
from concourse import bacc, mybir
from concourse.bass_utils import run_bass_kernel_spmd

F32 = mybir.dt.float32
BF16 = mybir.dt.bfloat16
AF = mybir.ActivationFunctionType
ALU = mybir.AluOpType

# Problem dims (hardcoded per contract)
B = 2
S = 2048
E = 2048
HQ = 16
HKV = 4
D = 128
REP = HQ // HKV          # 4 q-heads per kv head
NDEV = 8
DPB = 4                  # devices per batch
SQ = S // DPB            # 512 local query rows
EC = E // 128            # 16 contraction chunks
SKC = S // 128           # 16 key chunks
SCALE = 1.0 / float(np.sqrt(D))
VOFS = 2 * D * SQ        # bytes.. elems per region in a pair's collective buffer

_CACHE = {}


def _build(with_bias_qkv, with_bias_o):
    nc = bacc.Bacc("TRN2", target_bir_lowering=False, debug=False, num_devices=NDEV)

    xT = nc.dram_tensor("xT", [128, EC * SQ], BF16, kind="ExternalInput").ap()
    wq = nc.dram_tensor("wq", [128, HQ * EC * 128], BF16, kind="ExternalInput").ap()
    wk = nc.dram_tensor("wk", [128, HKV * EC * 128], BF16, kind="ExternalInput").ap()
    wv = nc.dram_tensor("wv", [128, HKV * EC * 128], BF16, kind="ExternalInput").ap()
    wo = nc.dram_tensor("wo", [128, HQ * E], BF16, kind="ExternalInput").ap()
    cosT = nc.dram_tensor("cosT", [D // 2, SQ], F32, kind="ExternalInput").ap()
    sinT = nc.dram_tensor("sinT", [D // 2, SQ], F32, kind="ExternalInput").ap()
    if with_bias_qkv:
        bqd = nc.dram_tensor("bqd", [D, HQ], F32, kind="ExternalInput").ap()
        bkd = nc.dram_tensor("bkd", [D, HKV], F32, kind="ExternalInput").ap()
        bvr = nc.dram_tensor("bvr", [1, HKV * D], F32, kind="ExternalInput").ap()
    if with_bias_o:
        bod = nc.dram_tensor("bod", [1, E], F32, kind="ExternalInput").ap()
    out = nc.dram_tensor("out", [SQ, E], F32, kind="ExternalOutput").ap()

    with tile.TileContext(nc) as tc, ExitStack() as es:
        _emit(tc, es, locals(), with_bias_qkv, with_bias_o)
    nc.compile()
    return nc


def _emit(tc, es, t, with_bias_qkv, with_bias_o):
    nc = tc.nc
    xT, wq, wk, wv, wo = t["xT"], t["wq"], t["wk"], t["wv"], t["wo"]
    cosT, sinT, out = t["cosT"], t["sinT"], t["out"]

    # ---------- persistent pools ----------
    const_pool = es.enter_context(tc.tile_pool(name="const", bufs=1))
    dram = es.enter_context(tc.tile_pool(name="dram", bufs=1, space="DRAM"))

    ones_f = const_pool.tile([128, 1], F32, tag="ones_f")
    nc.vector.memset(ones_f[:], 1.0)
    ones = const_pool.tile([128, 1], BF16, tag="ones")
    nc.vector.tensor_copy(ones[:], ones_f[:])

    if with_bias_qkv:
        bq_sb = const_pool.tile([D, HQ], F32, tag="bq")
        nc.sync.dma_start(bq_sb[:], t["bqd"])
        bk_sb = const_pool.tile([D, HKV], F32, tag="bk")
        nc.sync.dma_start(bk_sb[:], t["bkd"])
        bvr_sb = const_pool.tile([1, HKV * D], F32, tag="bvr")
        nc.sync.dma_start(bvr_sb[:], t["bvr"])
        bvb = const_pool.tile([128, HKV * D], F32, tag="bvb")
        nc.gpsimd.partition_broadcast(bvb[:], bvr_sb[:])
        bvbview = bvb[:].rearrange("p (h d) -> p h d", d=128)

    q_sb = const_pool.tile([128, HQ * SQ], BF16, tag="q_sb")        # q^T, rope'd
    attn_sb = const_pool.tile([128, HQ * SQ], BF16, tag="attn_sb")  # out^T per head

    # Flat per-pair collective buffer: [0:VOFS) = k^T as (hh, d, s);
    # [VOFS:2*VOFS) = v s-major as (s, hh, d).
    kv_sl = [dram.tile([2 * VOFS], BF16, tag=f"kvsl{p}", name=f"kvsl{p}") for p in range(2)]
    kv_g = [dram.tile([DPB, 2 * VOFS], BF16, tag=f"kvg{p}", name=f"kvg{p}") for p in range(2)]

    def rope(dst, src_ps, n_heads, cos_t, sin_t, tmp_pool, bias_sb=None, head0=0):
        """dst/src: [128, n_heads*SQ]; halves along partitions. bias optional."""
        w = n_heads * SQ
        src = src_ps[:].rearrange("p (h s) -> p h s", h=n_heads)
        if bias_sb is not None:
            for j in range(n_heads):
                nc.vector.tensor_scalar_add(
                    src_ps[:, j * SQ : (j + 1) * SQ],
                    src_ps[:, j * SQ : (j + 1) * SQ],
                    bias_sb[:, head0 + j : head0 + j + 1],
                )
        dstv = dst[:].rearrange("p (h s) -> p h s", h=n_heads)
        cosb = cos_t[:, None, :].to_broadcast((64, n_heads, SQ))
        sinb = sin_t[:, None, :].to_broadcast((64, n_heads, SQ))
        q1 = src[0:64]
        q2 = src[64:128]
        m1 = tmp_pool.tile([64, w], F32, tag="m", name="m1")[:].rearrange("p (h s) -> p h s", h=n_heads)
        m2 = tmp_pool.tile([64, w], F32, tag="m", name="m2")[:].rearrange("p (h s) -> p h s", h=n_heads)
        nc.vector.tensor_tensor(m1, q1, cosb, ALU.mult)
        nc.vector.tensor_tensor(m2, q2, sinb, ALU.mult)
        nc.vector.tensor_tensor(dstv[0:64], m1, m2, ALU.subtract)
        m3 = tmp_pool.tile([64, w], F32, tag="m", name="m3")[:].rearrange("p (h s) -> p h s", h=n_heads)
        m4 = tmp_pool.tile([64, w], F32, tag="m", name="m4")[:].rearrange("p (h s) -> p h s", h=n_heads)
        nc.vector.tensor_tensor(m3, q2, cosb, ALU.mult)
        nc.vector.tensor_tensor(m4, q1, sinb, ALU.mult)
        nc.vector.tensor_tensor(dstv[64:128], m3, m4, ALU.add)

    # kv_all is allocated BEFORE p12 so its SBUF range does not overlap the
    # projection-phase tiles: otherwise the gather-load DMAs inherit an
    # anti-dependency on the last wq_sb read and stall until Q-proj ends.
    kv_all = es.enter_context(tc.tile_pool(name="kv_all", bufs=1))
    k_all = kv_all.tile([128, HKV * S], BF16, tag="k_all")   # [d, h*S + sk]
    v_all = kv_all.tile([128, HKV * S], BF16, tag="v_all")   # [s%128, h*S + c*128 + d]

    # ---------- phase 1: per-pair KV projection + gather ----
    with (
        tc.tile_pool(name="p12", bufs=1) as p12,
        tc.tile_pool(name="proj_ps", bufs=3, space="PSUM") as proj_ps,
        tc.tile_pool(name="v_ps", bufs=2, space="PSUM") as v_ps,
        tc.tile_pool(name="rope_tmp", bufs=4) as rope_tmp,
    ):
        # Startup DMAs, ordered so the first K-proj matmul can issue ASAP.
        xT_sb = p12.tile([128, EC * SQ], BF16, tag="xT")
        wk_sb = p12.tile([128, HKV * EC * 128], BF16, tag="wk")
        wv_sb = p12.tile([128, HKV * EC * 128], BF16, tag="wv")
        PW = 2 * EC * 128  # one kv-head pair of weight columns
        nc.sync.dma_start(wk_sb[:, 0:PW], wk[:, 0:PW])
        nc.sync.dma_start(xT_sb[:], xT)
        nc.sync.dma_start(wv_sb[:, 0:PW], wv[:, 0:PW])
        cos_sb = p12.tile([64, SQ], F32, tag="cos")
        nc.sync.dma_start(cos_sb[:], cosT)
        sin_sb = p12.tile([64, SQ], F32, tag="sin")
        nc.sync.dma_start(sin_sb[:], sinT)
        nc.sync.dma_start(wk_sb[:, PW:], wk[:, PW:])
        nc.sync.dma_start(wv_sb[:, PW:], wv[:, PW:])
        wkview = wk_sb[:].rearrange("p (h c n) -> p h c n", c=EC, n=128)
        wvview = wv_sb[:].rearrange("p (h c n) -> p h c n", c=EC, n=128)
        xview = xT_sb[:].rearrange("p (c s) -> p c s", s=SQ)

        wq_sb = p12.tile([128, HQ * EC * 128], BF16, tag="wq")
        QW = 4 * EC * 128  # 4 heads per DMA
        for g4 in range(HQ // 4):
            nc.sync.dma_start(
                wq_sb[:, g4 * QW : (g4 + 1) * QW], wq[:, g4 * QW : (g4 + 1) * QW]
            )
        wqview = wq_sb[:].rearrange("p (h c n) -> p h c n", c=EC, n=128)

        k_sb = p12.tile([128, HKV * SQ], BF16, tag="k_sb")
        v_loc = p12.tile([128, HKV * 4 * 128], BF16, tag="v_loc")  # (h, sc, d)
        vlview = v_loc[:].rearrange("p (h c d) -> p h c d", c=4, d=128)
        for pair in range(2):
            # K projection (weight-stationary) + rope for kv heads {2p, 2p+1}
            ps = proj_ps.tile([128, 2 * SQ], F32, tag="proj")
            for j in range(2):
                h = pair * 2 + j
                for c in range(EC):
                    nc.tensor.matmul(
                        ps[:, j * SQ : (j + 1) * SQ],
                        wkview[:, h, c, :],
                        xview[:, c, :],
                        start=(c == 0),
                        stop=(c == EC - 1),
                    )
            rope(
                k_sb[:, pair * 2 * SQ : (pair + 1) * 2 * SQ],
                ps, 2, cos_sb, sin_sb, rope_tmp,
                bias_sb=(bk_sb if with_bias_qkv else None), head0=pair * 2,
            )
            # V projection x-stationary: out [s-block, (hh, d)] is s-major
            for sc in range(4):
                vps = v_ps.tile([128, 2 * 128], F32, tag="vps")
                vpsv = vps[:].rearrange("p (h d) -> p h d", d=128)
                for c in range(EC):
                    nc.tensor.matmul(
                        vpsv,
                        xview[:, c, sc * 128 : (sc + 1) * 128],
                        wvview[:, pair * 2 : pair * 2 + 2, c, :],
                        start=(c == 0),
                        stop=(c == EC - 1),
                    )
                if with_bias_qkv:
                    nc.vector.tensor_tensor(
                        vpsv, vpsv,
                        bvbview[:, pair * 2 : pair * 2 + 2, :], ALU.add,
                    )
                nc.vector.tensor_copy(
                    vlview[:, pair * 2 : pair * 2 + 2, sc, :], vpsv
                )
            # ship the pair's K and V to the collective buffer: 2 big DMAs
            kreg = kv_sl[pair][0:VOFS].rearrange("(h d s) -> d h s", h=2, s=SQ)
            nc.sync.dma_start(
                kreg,
                k_sb[:, pair * 2 * SQ : (pair + 1) * 2 * SQ].rearrange(
                    "p (h s) -> p h s", h=2
                ),
            )
            vreg = kv_sl[pair][VOFS : 2 * VOFS].rearrange(
                "(c p h d) -> p c h d", p=128, h=2, d=128
            )
            for hh in range(2):
                nc.sync.dma_start(
                    vreg[:, :, hh, :],
                    vlview[:, pair * 2 + hh, :, :],
                )
            nc.gpsimd.collective_compute(
                "AllGather",
                ALU.bypass,
                ins=[kv_sl[pair][:].opt()],
                outs=[kv_g[pair][:].opt()],
                replica_groups=[[0, 1, 2, 3], [4, 5, 6, 7]],
            )

        # ---------- phase 2: Q projection + rope (overlaps collectives) -----
        for g in range(HQ // 2):
            ps = proj_ps.tile([128, 2 * SQ], F32, tag="proj")
            for j in range(2):
                h = g * 2 + j
                for c in range(EC):
                    nc.tensor.matmul(
                        ps[:, j * SQ : (j + 1) * SQ],
                        wqview[:, h, c, :],
                        xview[:, c, :],
                        start=(c == 0),
                        stop=(c == EC - 1),
                    )
            rope(
                q_sb[:, g * 2 * SQ : (g + 1) * 2 * SQ],
                ps, 2, cos_sb, sin_sb, rope_tmp,
                bias_sb=(bq_sb if with_bias_qkv else None), head0=g * 2,
            )

    # ---------- phase 3: attention ----------
    with (
        tc.tile_pool(name="wo_pool", bufs=1) as wo_pool,
        ExitStack() as attn_es,
    ):
        # Wo resident for phase 4; streams during the collective wait.
        wo_sb = wo_pool.tile([128, HQ * E], BF16, tag="wo_sb")
        OW = 4 * E
        for g4 in range(HQ // 4):
            nc.sync.dma_start(
                wo_sb[:, g4 * OW : (g4 + 1) * OW], wo[:, g4 * OW : (g4 + 1) * OW]
            )
        woview = wo_sb[:].rearrange("p (h e) -> p h e", e=E)

        sc_ps = attn_es.enter_context(tc.tile_pool(name="sc_ps", bufs=2, space="PSUM"))
        out_ps = attn_es.enter_context(tc.tile_pool(name="out_ps", bufs=1, space="PSUM"))
        sum_ps = attn_es.enter_context(tc.tile_pool(name="sum_ps", bufs=2, space="PSUM"))
        p_pool = attn_es.enter_context(tc.tile_pool(name="p_pool", bufs=4))
        den_pool = attn_es.enter_context(tc.tile_pool(name="den_pool", bufs=3))

        # full-sequence K^T and V per kv head
        for pair in range(2):
            for si in range(DPB):
                nc.sync.dma_start(
                    k_all[:, :].rearrange("p (h s) -> p h s", h=HKV)[
                        :, pair * 2 : pair * 2 + 2, si * SQ : (si + 1) * SQ
                    ],
                    kv_g[pair][si, 0:VOFS].rearrange("(h d s) -> d h s", h=2, s=SQ),
                )
                for hh in range(2):
                    nc.sync.dma_start(
                        v_all[:, :].rearrange("p (h c d) -> p h c d", h=HKV, d=128)[
                            :, pair * 2 + hh, si * 4 : (si + 1) * 4, :
                        ],
                        kv_g[pair][si, VOFS : 2 * VOFS].rearrange(
                            "(c p h d) -> p c h d", p=128, h=2, d=128
                        )[:, :, hh, :],
                    )

        for hp in range(HQ // 2):      # head pair: q-heads {2hp, 2hp+1}
            kh = hp // 2               # shared kv head
            h0 = 2 * hp
            op = out_ps.tile([128, 2 * SQ], F32, tag="outp")
            sps = [
                sum_ps.tile([1, SQ], F32, tag="sump", name=f"sp{j}") for j in range(2)
            ]
            pts = [None] * SKC

            def emit_av(c):
                vchunk = v_all[:, kh * S + c * 128 : kh * S + (c + 1) * 128]
                for j in range(2):
                    nc.tensor.matmul(
                        op[:, j * SQ : (j + 1) * SQ],
                        vchunk,
                        pts[c][:, j * SQ : (j + 1) * SQ],
                        start=(c == 0),
                        stop=(c == SKC - 1),
                        skip_group_check=True,
                    )
                for j in range(2):
                    nc.tensor.matmul(
                        sps[j][:],
                        ones[:],
                        pts[c][:, j * SQ : (j + 1) * SQ],
                        start=(c == 0),
                        stop=(c == SKC - 1),
                        skip_group_check=True,
                    )

            for c in range(SKC):
                kchunk = k_all[:, kh * S + c * 128 : kh * S + (c + 1) * 128]
                scp = sc_ps.tile([128, 2 * SQ], F32, tag="scp")
                for j in range(2):
                    nc.tensor.matmul(
                        scp[:, j * SQ : (j + 1) * SQ],
                        kchunk,
                        q_sb[:, (h0 + j) * SQ : (h0 + j + 1) * SQ],
                        start=True,
                        stop=True,
                    )
                pt = p_pool.tile([128, 2 * SQ], BF16, tag="pt")
                nc.scalar.activation(pt[:], scp[:], AF.Exp, scale=SCALE)
                pts[c] = pt
                if c >= 1:
                    emit_av(c - 1)
            emit_av(SKC - 1)

            # drain unnormalized out^T (frees op quickly), copy den sums out of
            # PSUM (frees sp banks), then normalize in SBUF
            dcs = []
            for j in range(2):
                dc = den_pool.tile([1, SQ], F32, tag="dc", name=f"dc{j}")
                nc.vector.tensor_copy(dc[:], sps[j][:])
                dcs.append(dc)
            un = attn_sb[:, h0 * SQ : (h0 + 2) * SQ]
            nc.vector.tensor_copy(un, op[:])
            for j in range(2):
                h = h0 + j
                rs = den_pool.tile([1, SQ], F32, tag="rs", name=f"rs{j}")
                nc.vector.reciprocal(rs[:], dcs[j][:])
                den = den_pool.tile([128, SQ], F32, tag="den", name=f"den{j}")
                nc.gpsimd.partition_broadcast(den[:], rs[:])
                nc.vector.tensor_tensor(
                    attn_sb[:, h * SQ : (h + 1) * SQ],
                    attn_sb[:, h * SQ : (h + 1) * SQ],
                    den[:], ALU.mult,
                )

        # ---------- phase 4: o_proj ----------
        attn_es.close()
        with (
            tc.tile_pool(name="o_ps", bufs=2, space="PSUM") as o_ps,
            tc.tile_pool(name="o_sb", bufs=3) as o_sb_pool,
        ):
            if with_bias_o:
                bo_sb = const_pool.tile([1, E], F32, tag="bo")
                nc.sync.dma_start(bo_sb[:], t["bod"])
                bo_b = const_pool.tile([128, E], F32, tag="bo_b")
                nc.gpsimd.partition_broadcast(bo_b[:], bo_sb[:])
            # sqc-outer so each attn_sb chunk is the stationary operand for
            # 4 consecutive matmuls (one per output-column tile)
            for sqc in range(SQ // 128):
                pss = [
                    o_ps.tile([128, 512], F32, tag=f"ops{et}", name=f"ops{et}")
                    for et in range(4)
                ]
                for hd in range(HQ):
                    for et in range(4):
                        nc.tensor.matmul(
                            pss[et][:],
                            attn_sb[:, hd * SQ + sqc * 128 : hd * SQ + (sqc + 1) * 128],
                            woview[:, hd, et * 512 : (et + 1) * 512],
                            start=(hd == 0),
                            stop=(hd == HQ - 1),
                        )
                for et in range(4):
                    ot = o_sb_pool.tile([128, 512], F32, tag="osb")
                    if with_bias_o:
                        nc.vector.tensor_tensor(
                            ot[:], pss[et][:], bo_b[:, et * 512 : (et + 1) * 512], ALU.add
                        )
                    else:
                        nc.scalar.copy(ot[:], pss[et][:])
                    nc.sync.dma_start(
                        out[sqc * 128 : (sqc + 1) * 128, et * 512 : (et + 1) * 512],
                        ot[:],
                    )


RUN_KWARGS = {}


def kernel(x, sin, cos, Wq, bq, Wk, bk, Wv, bv, Wo, bo, sinks):
    x = np.asarray(x, dtype=np.float32)
    sin = np.asarray(sin, dtype=np.float32)
    cos = np.asarray(cos, dtype=np.float32)
    with_bias_qkv = bool(np.any(bq) or np.any(bk) or np.any(bv))
    with_bias_o = bool(np.any(bo))

    key = (with_bias_qkv, with_bias_o)
    if key not in _CACHE:
        _CACHE[key] = _build(with_bias_qkv, with_bias_o)
    nc = _CACHE[key]

    def tile_w(W, H):
        # [E, H*128] -> [128, H*EC*128] with free index (h, c, n)
        W = np.asarray(W, dtype=np.float32)
        return np.ascontiguousarray(
            W.reshape(EC, 128, H, 128).transpose(1, 2, 0, 3).reshape(128, H * EC * 128)
        ).astype(bfloat16)

    wq_t = tile_w(Wq, HQ)
    wk_t = tile_w(Wk, HKV)
    wv_t = tile_w(Wv, HKV)
    # Wo [HQ*D, E] -> [128, HQ*E] with free index (hd, e)
    wo_t = np.ascontiguousarray(
        np.asarray(Wo, np.float32).reshape(HQ, 128, E).transpose(1, 0, 2).reshape(128, HQ * E)
    ).astype(bfloat16)

    in_maps = []
    for dev in range(NDEV):
        b, i = divmod(dev, DPB)
        sl = slice(SQ * i, SQ * (i + 1))
        xs = x[b, sl, :]  # [SQ, E]
        xT_t = np.ascontiguousarray(
            xs.T.reshape(EC, 128, SQ).transpose(1, 0, 2).reshape(128, EC * SQ)
        ).astype(bfloat16)
        m = {
            "xT": xT_t,
            "wq": wq_t,
            "wk": wk_t,
            "wv": wv_t,
            "wo": wo_t,
            "cosT": np.ascontiguousarray(cos[b, sl, :].T),
            "sinT": np.ascontiguousarray(sin[b, sl, :].T),
        }
        if with_bias_qkv:
            m["bqd"] = np.ascontiguousarray(np.asarray(bq, np.float32).reshape(HQ, D).T)
            m["bkd"] = np.ascontiguousarray(np.asarray(bk, np.float32).reshape(HKV, D).T)
            m["bvr"] = np.asarray(bv, np.float32).reshape(1, HKV * D)
        if with_bias_o:
            m["bod"] = np.asarray(bo, np.float32).reshape(1, E)
        in_maps.append(m)

    res = run_bass_kernel_spmd(nc, in_maps, list(range(NDEV)), **RUN_KWARGS)
    kernel.last_result = res

    out = np.empty((B, S, E), dtype=np.float32)
    for dev in range(NDEV):
        b, i = divmod(dev, DPB)
        out[b, SQ * i : SQ * (i + 1), :] = res.results[dev]["out"]
    return out


# revision 23
# speedup vs baseline: 1.1121x; 1.0433x over previous
"""GroupedQueryAttention Bass kernel for 8 Trainium2 NeuronCores.

Sharding: 8 devices = 2 batches x 4 sequence-quarters.
Device d handles batch b=d//4, query rows [512*i, 512*(i+1)) with i=d%4.

v4: all matmul operands BF16 (FastWeightLoad + warm HAM clock; the fp32r
baseline ran the PE at 1.2GHz).  Weights pre-tiled host-side into contiguous
layouts (few large DMAs).  V projection runs x-stationary so V comes out
s-major directly -- no PE transposes, and each kv-head pair's K/V slice
ships to the collective buffer as 2 large DMAs.  The KV AllGather is split
into two per-pair collectives so attention on q-heads 0-7 only waits for the
first one.  Attention processes q-heads in pairs sharing one kv head: the
k/v chunk is the stationary operand for both heads' matmuls, exp is batched
[128,1024] per chunk with no bias (the additive 'sinks' term is constant
per head across q AND k, so softmax cancels it exactly), and both heads'
softmax denominators accumulate in one PSUM bank (partitions 0/32) so one
reciprocal serves the pair.  Normalization happens in SBUF after an
unnormalized bf16 drain, keeping PSUM pressure at 8 banks exactly.
"""

from contextlib import ExitStack

import numpy as np
from ml_dtypes import bfloat16

import concourse.bass as bass
import concourse.tile as tile
from concourse import bacc, mybir
from concourse.bass_utils import run_bass_kernel_spmd

F32 = mybir.dt.float32
BF16 = mybir.dt.bfloat16
AF = mybir.ActivationFunctionType
ALU = mybir.AluOpType

# Problem dims (hardcoded per contract)
B = 2
S = 2048
E = 2048
HQ = 16
HKV = 4
D = 128
REP = HQ // HKV          # 4 q-heads per kv head
NDEV = 8
DPB = 4                  # devices per batch
SQ = S // DPB            # 512 local query rows
EC = E // 128            # 16 contraction chunks
SKC = S // 128           # 16 key chunks
SCALE = 1.0 / float(np.sqrt(D))
VOFS = 2 * D * SQ        # bytes.. elems per region in a pair's collective buffer

_CACHE = {}


def _build(with_bias_qkv, with_bias_o):
    nc = bacc.Bacc("TRN2", target_bir_lowering=False, debug=False, num_devices=NDEV)

    xT = nc.dram_tensor("xT", [128, EC * SQ], BF16, kind="ExternalInput").ap()
    wq = nc.dram_tensor("wq", [128, HQ * EC * 128], BF16, kind="ExternalInput").ap()
    wk = nc.dram_tensor("wk", [128, HKV * EC * 128], BF16, kind="ExternalInput").ap()
    wv = nc.dram_tensor("wv", [128, HKV * EC * 128], BF16, kind="ExternalInput").ap()
    wo = nc.dram_tensor("wo", [128, HQ * E], BF16, kind="ExternalInput").ap()
    cosT = nc.dram_tensor("cosT", [D // 2, SQ], F32, kind="ExternalInput").ap()
    sinT = nc.dram_tensor("sinT", [D // 2, SQ], F32, kind="ExternalInput").ap()
    if with_bias_qkv:
        bqd = nc.dram_tensor("bqd", [D, HQ], F32, kind="ExternalInput").ap()
        bkd = nc.dram_tensor("bkd", [D, HKV], F32, kind="ExternalInput").ap()
        bvr = nc.dram_tensor("bvr", [1, HKV * D], F32, kind="ExternalInput").ap()
    if with_bias_o:
        bod = nc.dram_tensor("bod", [1, E], F32, kind="ExternalInput").ap()
    out = nc.dram_tensor("out", [SQ, E], F32, kind="ExternalOutput").ap()

    with tile.TileContext(nc) as tc, ExitStack() as es:
        _emit(tc, es, locals(), with_bias_qkv, with_bias_o)
    nc.compile()
    return nc


def _emit(tc, es, t, with_bias_qkv, with_bias_o):
    nc = tc.nc
    xT, wq, wk, wv, wo = t["xT"], t["wq"], t["wk"], t["wv"], t["wo"]
    cosT, sinT, out = t["cosT"], t["sinT"], t["out"]

    # ---------- persistent pools ----------
    const_pool = es.enter_context(tc.tile_pool(name="const", bufs=1))
    dram = es.enter_context(tc.tile_pool(name="dram", bufs=1, space="DRAM"))

    ones_f = const_pool.tile([128, 1], F32, tag="ones_f")
    nc.vector.memset(ones_f[:], 1.0)
    ones = const_pool.tile([128, 1], BF16, tag="ones")
    nc.vector.tensor_copy(ones[:], ones_f[:])

    if with_bias_qkv:
        bq_sb = const_pool.tile([D, HQ], F32, tag="bq")
        nc.sync.dma_start(bq_sb[:], t["bqd"])
        bk_sb = const_pool.tile([D, HKV], F32, tag="bk")
        nc.sync.dma_start(bk_sb[:], t["bkd"])
        bvr_sb = const_pool.tile([1, HKV * D], F32, tag="bvr")
        nc.sync.dma_start(bvr_sb[:], t["bvr"])
        bvb = const_pool.tile([128, HKV * D], F32, tag="bvb")
        nc.gpsimd.partition_broadcast(bvb[:], bvr_sb[:])
        bvbview = bvb[:].rearrange("p (h d) -> p h d", d=128)

    q_sb = const_pool.tile([128, HQ * SQ], BF16, tag="q_sb")        # q^T, rope'd
    attn_sb = const_pool.tile([128, HQ * SQ], BF16, tag="attn_sb")  # out^T per head

    # Flat per-pair collective buffer: [0:VOFS) = k^T as (hh, d, s);
    # [VOFS:2*VOFS) = v s-major as (s, hh, d).
    kv_sl = [dram.tile([2 * VOFS], BF16, tag=f"kvsl{p}", name=f"kvsl{p}") for p in range(2)]
    kv_g = [dram.tile([DPB, 2 * VOFS], BF16, tag=f"kvg{p}", name=f"kvg{p}") for p in range(2)]

    def rope(dst, src_ps, n_heads, cos_t, sin_t, tmp_pool, bias_sb=None, head0=0):
        """dst/src: [128, n_heads*SQ]; halves along partitions. bias optional."""
        w = n_heads * SQ
        src = src_ps[:].rearrange("p (h s) -> p h s", h=n_heads)
        if bias_sb is not None:
            for j in range(n_heads):
                nc.vector.tensor_scalar_add(
                    src_ps[:, j * SQ : (j + 1) * SQ],
                    src_ps[:, j * SQ : (j + 1) * SQ],
                    bias_sb[:, head0 + j : head0 + j + 1],
                )
        dstv = dst[:].rearrange("p (h s) -> p h s", h=n_heads)
        cosb = cos_t[:, None, :].to_broadcast((64, n_heads, SQ))
        sinb = sin_t[:, None, :].to_broadcast((64, n_heads, SQ))
        q1 = src[0:64]
        q2 = src[64:128]
        m1 = tmp_pool.tile([64, w], F32, tag="m", name="m1")[:].rearrange("p (h s) -> p h s", h=n_heads)
        m2 = tmp_pool.tile([64, w], F32, tag="m", name="m2")[:].rearrange("p (h s) -> p h s", h=n_heads)
        nc.vector.tensor_tensor(m1, q1, cosb, ALU.mult)
        nc.vector.tensor_tensor(m2, q2, sinb, ALU.mult)
        nc.vector.tensor_tensor(dstv[0:64], m1, m2, ALU.subtract)
        m3 = tmp_pool.tile([64, w], F32, tag="m", name="m3")[:].rearrange("p (h s) -> p h s", h=n_heads)
        m4 = tmp_pool.tile([64, w], F32, tag="m", name="m4")[:].rearrange("p (h s) -> p h s", h=n_heads)
        nc.vector.tensor_tensor(m3, q2, cosb, ALU.mult)
        nc.vector.tensor_tensor(m4, q1, sinb, ALU.mult)
        nc.vector.tensor_tensor(dstv[64:128], m3, m4, ALU.add)

    # kv_all is allocated BEFORE p12 so its SBUF range does not overlap the
    # projection-phase tiles: otherwise the gather-load DMAs inherit an
    # anti-dependency on the last wq_sb read and stall until Q-proj ends.
    kv_all = es.enter_context(tc.tile_pool(name="kv_all", bufs=1))
    k_all = kv_all.tile([128, HKV * S], BF16, tag="k_all")   # [d, h*S + sk]
    v_all = kv_all.tile([128, HKV * S], BF16, tag="v_all")   # [s%128, h*S + c*128 + d]

    # ---------- phase 1: per-pair KV projection + gather ----
    with (
        tc.tile_pool(name="p12", bufs=1) as p12,
        tc.tile_pool(name="proj_ps", bufs=3, space="PSUM") as proj_ps,
        tc.tile_pool(name="v_ps", bufs=2, space="PSUM") as v_ps,
        tc.tile_pool(name="rope_tmp", bufs=4) as rope_tmp,
    ):
        # Startup DMAs, ordered so the first K-proj matmul can issue ASAP.
        xT_sb = p12.tile([128, EC * SQ], BF16, tag="xT")
        wk_sb = p12.tile([128, HKV * EC * 128], BF16, tag="wk")
        wv_sb = p12.tile([128, HKV * EC * 128], BF16, tag="wv")
        PW = 2 * EC * 128  # one kv-head pair of weight columns
        nc.sync.dma_start(xT_sb[:], xT)
        nc.sync.dma_start(wk_sb[:, 0:PW], wk[:, 0:PW])
        nc.sync.dma_start(wv_sb[:, 0:PW], wv[:, 0:PW])
        cos_sb = p12.tile([64, SQ], F32, tag="cos")
        nc.sync.dma_start(cos_sb[:], cosT)
        sin_sb = p12.tile([64, SQ], F32, tag="sin")
        nc.sync.dma_start(sin_sb[:], sinT)
        nc.sync.dma_start(wk_sb[:, PW:], wk[:, PW:])
        nc.sync.dma_start(wv_sb[:, PW:], wv[:, PW:])
        wkview = wk_sb[:].rearrange("p (h c n) -> p h c n", c=EC, n=128)
        wvview = wv_sb[:].rearrange("p (h c n) -> p h c n", c=EC, n=128)
        xview = xT_sb[:].rearrange("p (c s) -> p c s", s=SQ)

        wq_sb = p12.tile([128, HQ * EC * 128], BF16, tag="wq")
        QW = 4 * EC * 128  # 4 heads per DMA
        for g4 in range(HQ // 4):
            nc.sync.dma_start(
                wq_sb[:, g4 * QW : (g4 + 1) * QW], wq[:, g4 * QW : (g4 + 1) * QW]
            )
        wqview = wq_sb[:].rearrange("p (h c n) -> p h c n", c=EC, n=128)

        k_sb = p12.tile([128, HKV * SQ], BF16, tag="k_sb")
        v_loc = p12.tile([128, HKV * 4 * 128], BF16, tag="v_loc")  # (h, sc, d)
        vlview = v_loc[:].rearrange("p (h c d) -> p h c d", c=4, d=128)
        for pair in range(2):
            # K projection (weight-stationary) + rope for kv heads {2p, 2p+1}
            ps = proj_ps.tile([128, 2 * SQ], F32, tag="proj")
            for j in range(2):
                h = pair * 2 + j
                for c in range(EC):
                    nc.tensor.matmul(
                        ps[:, j * SQ : (j + 1) * SQ],
                        wkview[:, h, c, :],
                        xview[:, c, :],
                        start=(c == 0),
                        stop=(c == EC - 1),
                    )
            rope(
                k_sb[:, pair * 2 * SQ : (pair + 1) * 2 * SQ],
                ps, 2, cos_sb, sin_sb, rope_tmp,
                bias_sb=(bk_sb if with_bias_qkv else None), head0=pair * 2,
            )
            # V projection x-stationary: out [s-block, (hh, d)] is s-major
            for sc in range(4):
                vps = v_ps.tile([128, 2 * 128], F32, tag="vps")
                vpsv = vps[:].rearrange("p (h d) -> p h d", d=128)
                for c in range(EC):
                    nc.tensor.matmul(
                        vpsv,
                        xview[:, c, sc * 128 : (sc + 1) * 128],
                        wvview[:, pair * 2 : pair * 2 + 2, c, :],
                        start=(c == 0),
                        stop=(c == EC - 1),
                    )
                if with_bias_qkv:
                    nc.vector.tensor_tensor(
                        vpsv, vpsv,
                        bvbview[:, pair * 2 : pair * 2 + 2, :], ALU.add,
                    )
                nc.vector.tensor_copy(
                    vlview[:, pair * 2 : pair * 2 + 2, sc, :], vpsv
                )
            # ship the pair's K and V to the collective buffer: 2 big DMAs
            kreg = kv_sl[pair][0:VOFS].rearrange("(h d s) -> d h s", h=2, s=SQ)
            nc.sync.dma_start(
                kreg,
                k_sb[:, pair * 2 * SQ : (pair + 1) * 2 * SQ].rearrange(
                    "p (h s) -> p h s", h=2
                ),
            )
            vreg = kv_sl[pair][VOFS : 2 * VOFS].rearrange(
                "(c p h d) -> p c h d", p=128, h=2, d=128
            )
            for hh in range(2):
                nc.sync.dma_start(
                    vreg[:, :, hh, :],
                    vlview[:, pair * 2 + hh, :, :],
                )
            nc.gpsimd.collective_compute(
                "AllGather",
                ALU.bypass,
                ins=[kv_sl[pair][:].opt()],
                outs=[kv_g[pair][:].opt()],
                replica_groups=[[0, 1, 2, 3], [4, 5, 6, 7]],
            )

        # ---------- phase 2: Q projection + rope (overlaps collectives) -----
        for g in range(HQ // 2):
            ps = proj_ps.tile([128, 2 * SQ], F32, tag="proj")
            for j in range(2):
                h = g * 2 + j
                for c in range(EC):
                    nc.tensor.matmul(
                        ps[:, j * SQ : (j + 1) * SQ],
                        wqview[:, h, c, :],
                        xview[:, c, :],
                        start=(c == 0),
                        stop=(c == EC - 1),
                    )
            rope(
                q_sb[:, g * 2 * SQ : (g + 1) * 2 * SQ],
                ps, 2, cos_sb, sin_sb, rope_tmp,
                bias_sb=(bq_sb if with_bias_qkv else None), head0=g * 2,
            )

    # ---------- phase 3: attention ----------
    with (
        tc.tile_pool(name="wo_pool", bufs=1) as wo_pool,
        ExitStack() as attn_es,
    ):
        sc_ps = attn_es.enter_context(tc.tile_pool(name="sc_ps", bufs=2, space="PSUM"))
        out_ps = attn_es.enter_context(tc.tile_pool(name="out_ps", bufs=1, space="PSUM"))
        sum_ps = attn_es.enter_context(tc.tile_pool(name="sum_ps", bufs=2, space="PSUM"))
        p_pool = attn_es.enter_context(tc.tile_pool(name="p_pool", bufs=4))
        den_pool = attn_es.enter_context(tc.tile_pool(name="den_pool", bufs=3))

        # full-sequence K^T and V per kv head.  These loads precede the Wo
        # transfer in the DMA queue so attention isn't starved behind 8MB of
        # o_proj weights (Wo isn't read until the very end).
        for pair in range(2):
            for si in range(DPB):
                nc.sync.dma_start(
                    k_all[:, :].rearrange("p (h s) -> p h s", h=HKV)[
                        :, pair * 2 : pair * 2 + 2, si * SQ : (si + 1) * SQ
                    ],
                    kv_g[pair][si, 0:VOFS].rearrange("(h d s) -> d h s", h=2, s=SQ),
                )
                for hh in range(2):
                    nc.sync.dma_start(
                        v_all[:, :].rearrange("p (h c d) -> p h c d", h=HKV, d=128)[
                            :, pair * 2 + hh, si * 4 : (si + 1) * 4, :
                        ],
                        kv_g[pair][si, VOFS : 2 * VOFS].rearrange(
                            "(c p h d) -> p c h d", p=128, h=2, d=128
                        )[:, :, hh, :],
                    )

        wo_sb = wo_pool.tile([128, HQ * E], BF16, tag="wo_sb")
        OW = 4 * E
        for g4 in range(HQ // 4):
            nc.sync.dma_start(
                wo_sb[:, g4 * OW : (g4 + 1) * OW], wo[:, g4 * OW : (g4 + 1) * OW]
            )
        woview = wo_sb[:].rearrange("p (h e) -> p h e", e=E)

        for hp in range(HQ // 2):      # head pair: q-heads {2hp, 2hp+1}
            kh = hp // 2               # shared kv head
            h0 = 2 * hp
            op = out_ps.tile([128, 2 * SQ], F32, tag="outp")
            sps = [
                sum_ps.tile([1, SQ], F32, tag="sump", name=f"sp{j}") for j in range(2)
            ]
            pts = [None] * SKC

            def emit_av(c):
                vchunk = v_all[:, kh * S + c * 128 : kh * S + (c + 1) * 128]
                for j in range(2):
                    nc.tensor.matmul(
                        op[:, j * SQ : (j + 1) * SQ],
                        vchunk,
                        pts[c][:, j * SQ : (j + 1) * SQ],
                        start=(c == 0),
                        stop=(c == SKC - 1),
                        skip_group_check=True,
                    )
                for j in range(2):
                    nc.tensor.matmul(
                        sps[j][:],
                        ones[:],
                        pts[c][:, j * SQ : (j + 1) * SQ],
                        start=(c == 0),
                        stop=(c == SKC - 1),
                        skip_group_check=True,
                    )

            for c in range(SKC):
                kchunk = k_all[:, kh * S + c * 128 : kh * S + (c + 1) * 128]
                scp = sc_ps.tile([128, 2 * SQ], F32, tag="scp")
                for j in range(2):
                    nc.tensor.matmul(
                        scp[:, j * SQ : (j + 1) * SQ],
                        kchunk,
                        q_sb[:, (h0 + j) * SQ : (h0 + j + 1) * SQ],
                        start=True,
                        stop=True,
                    )
                pt = p_pool.tile([128, 2 * SQ], BF16, tag="pt")
                nc.scalar.activation(pt[:], scp[:], AF.Exp, scale=SCALE)
                pts[c] = pt
                if c >= 1:
                    emit_av(c - 1)
            emit_av(SKC - 1)

            # drain unnormalized out^T (frees op quickly), copy den sums out of
            # PSUM (frees sp banks), then normalize in SBUF
            dcs = []
            for j in range(2):
                dc = den_pool.tile([1, SQ], F32, tag="dc", name=f"dc{j}")
                nc.vector.tensor_copy(dc[:], sps[j][:])
                dcs.append(dc)
            un = attn_sb[:, h0 * SQ : (h0 + 2) * SQ]
            nc.vector.tensor_copy(un, op[:])
            for j in range(2):
                h = h0 + j
                rs = den_pool.tile([1, SQ], F32, tag="rs", name=f"rs{j}")
                nc.vector.reciprocal(rs[:], dcs[j][:])
                den = den_pool.tile([128, SQ], F32, tag="den", name=f"den{j}")
                nc.gpsimd.partition_broadcast(den[:], rs[:])
                nc.vector.tensor_tensor(
                    attn_sb[:, h * SQ : (h + 1) * SQ],
                    attn_sb[:, h * SQ : (h + 1) * SQ],
                    den[:], ALU.mult,
                )

        # ---------- phase 4: o_proj ----------
        attn_es.close()
        with (
            tc.tile_pool(name="o_ps", bufs=2, space="PSUM") as o_ps,
            tc.tile_pool(name="o_sb", bufs=3) as o_sb_pool,
        ):
            if with_bias_o:
                bo_sb = const_pool.tile([1, E], F32, tag="bo")
                nc.sync.dma_start(bo_sb[:], t["bod"])
                bo_b = const_pool.tile([128, E], F32, tag="bo_b")
                nc.gpsimd.partition_broadcast(bo_b[:], bo_sb[:])
            # sqc-outer so each attn_sb chunk is the stationary operand for
            # 4 consecutive matmuls (one per output-column tile)
            for sqc in range(SQ // 128):
                pss = [
                    o_ps.tile([128, 512], F32, tag=f"ops{et}", name=f"ops{et}")
                    for et in range(4)
                ]
                for hd in range(HQ):
                    for et in range(4):
                        nc.tensor.matmul(
                            pss[et][:],
                            attn_sb[:, hd * SQ + sqc * 128 : hd * SQ + (sqc + 1) * 128],
                            woview[:, hd, et * 512 : (et + 1) * 512],
                            start=(hd == 0),
                            stop=(hd == HQ - 1),
                        )
                for et in range(4):
                    ot = o_sb_pool.tile([128, 512], F32, tag="osb")
                    if with_bias_o:
                        nc.vector.tensor_tensor(
                            ot[:], pss[et][:], bo_b[:, et * 512 : (et + 1) * 512], ALU.add
                        )
                    else:
                        nc.scalar.copy(ot[:], pss[et][:])
                    nc.sync.dma_start(
                        out[sqc * 128 : (sqc + 1) * 128, et * 512 : (et + 1) * 512],
                        ot[:],
                    )


RUN_KWARGS = {}


def kernel(x, sin, cos, Wq, bq, Wk, bk, Wv, bv, Wo, bo, sinks):
    x = np.asarray(x, dtype=np.float32)
    sin = np.asarray(sin, dtype=np.float32)
    cos = np.asarray(cos, dtype=np.float32)
    with_bias_qkv = bool(np.any(bq) or np.any(bk) or np.any(bv))
    with_bias_o = bool(np.any(bo))

    key = (with_bias_qkv, with_bias_o)
    if key not in _CACHE:
        _CACHE[key] = _build(with_bias_qkv, with_bias_o)
    nc = _CACHE[key]

    def tile_w(W, H):
        # [E, H*128] -> [128, H*EC*128] with free index (h, c, n)
        W = np.asarray(W, dtype=np.float32)
        return np.ascontiguousarray(
            W.reshape(EC, 128, H, 128).transpose(1, 2, 0, 3).reshape(128, H * EC * 128)
        ).astype(bfloat16)

    wq_t = tile_w(Wq, HQ)
    wk_t = tile_w(Wk, HKV)
    wv_t = tile_w(Wv, HKV)
    # Wo [HQ*D, E] -> [128, HQ*E] with free index (hd, e)
    wo_t = np.ascontiguousarray(
        np.asarray(Wo, np.float32).reshape(HQ, 128, E).transpose(1, 0, 2).reshape(128, HQ * E)
    ).astype(bfloat16)

    in_maps = []
    for dev in range(NDEV):
        b, i = divmod(dev, DPB)
        sl = slice(SQ * i, SQ * (i + 1))
        xs = x[b, sl, :]  # [SQ, E]
        xT_t = np.ascontiguousarray(
            xs.T.reshape(EC, 128, SQ).transpose(1, 0, 2).reshape(128, EC * SQ)
        ).astype(bfloat16)
        m = {
            "xT": xT_t,
            "wq": wq_t,
            "wk": wk_t,
            "wv": wv_t,
            "wo": wo_t,
            "cosT": np.ascontiguousarray(cos[b, sl, :].T),
            "sinT": np.ascontiguousarray(sin[b, sl, :].T),
        }
        if with_bias_qkv:
            m["bqd"] = np.ascontiguousarray(np.asarray(bq, np.float32).reshape(HQ, D).T)
            m["bkd"] = np.ascontiguousarray(np.asarray(bk, np.float32).reshape(HKV, D).T)
            m["bvr"] = np.asarray(bv, np.float32).reshape(1, HKV * D)
        if with_bias_o:
            m["bod"] = np.asarray(bo, np.float32).reshape(1, E)
        in_maps.append(m)

    res = run_bass_kernel_spmd(nc, in_maps, list(range(NDEV)), **RUN_KWARGS)
    kernel.last_result = res

    out = np.empty((B, S, E), dtype=np.float32)
    for dev in range(NDEV):
        b, i = divmod(dev, DPB)
        out[b, SQ * i : SQ * (i + 1), :] = res.results[dev]["out"]
    return out
